# revision 15
# baseline (speedup 1.0000x reference)
"""Grouped (MoE-routed) GEMM on 8 Trainium2 NeuronCores.

out[m, n] = sum_k lhs[m, k] * rhs[g[m], n, k],  g = clamp(m_indices, 0, G)

Strategy: expert-parallel. Host dispatches rows by m_indices (the
"all-to-all" is a host-side gather since we hold full inputs), core c gets
expert c's rows padded to a common M_pad, plus expert c's weight matrix.
Every core runs one identical dense GEMM program computing the transposed
output  oT[N, M_pad] = B[N, K] @ A[M_pad, K]^T  (bf16 in, fp32 acc).

v2 layout/ordering notes (driven by the v1 NTFF trace):
- All DRAM operands are partition-major so every load/store is ONE big
  3D descriptor (256KB-1MB). v1's 64-114KB descriptors serialized on the
  two HWDGE rings at ~150-250GB/s and starved the PE twice (a 3.4us idle
  that also re-throttled the HAM clock gate, costing ~5us more).
- B is nt-major in DRAM so B n-tile groups arrive in exactly the order
  the (m-chunk outer, n-tile inner) chain sweep consumes them.
- PSUM is managed as two 4-bank tiles [128, 4, 512]f32; a quad of 4
  chains shares one tile, then a single DVE/ACT cast moves all 4 banks
  to SBUF and a single DMA stores them. This quarters the cast/store
  instruction count: the end-of-NEFF teardown clears every semaphore
  ever allocated, one EVENT_SEMAPHORE each (~110ns serialized/engine),
  so instruction/semaphore count is ~6us of the measured window.
- The final quad casts/stores per-chain on the (idle by then) HWDGE
  rings to minimize the drain tail after the last matmul.
- 8 junk matmuls on a scratch tile lift the HAM clock gate (1.2->2.4GHz)
  during the load lead-in so the first real chain runs warm.
"""

import numpy as np
import ml_dtypes

K = 1024
N = 2048
G = 8
N_CORES = 8
KP = 128           # SBUF partitions / contraction chunk
KC = K // KP       # 8 k-chunks
NT = N // KP       # 16 stationary n-tiles
MCH = 512          # max moving-operand m-chunk (one PSUM bank of fp32)
N_WARMUP = 12      # dummy matmuls to lift the HAM clock gate

# B n-tile load groups with explicit ring assignment (0=sync, 1=scalar).
# Each HWDGE ring drains its descriptors FIFO at ~179GB/s (the two rings
# split the ~358GB/s HBM-per-core limit), so this list IS the flow
# control: a group must sit early enough in its ring that it lands
# before the (m-chunk outer, nt inner) sweep reaches it. Singles first
# (latency), pairs after (efficiency).
BT_GROUPS = [(0, 1, 0), (1, 2, 1), (2, 3, 0), (3, 4, 1),
             (4, 6, 0), (6, 8, 1), (8, 10, 0), (10, 12, 1),
             (12, 14, 0), (14, 16, 1)]

_BUILD_CACHE = {}


def _m_chunks(m_pad):
    """Near-equal 64-aligned chunks of <=512, smallest first and last.

    Small first chunk -> the first chain needs the least data in SBUF;
    small last chunk -> the final casts/stores drain quickly.
    """
    n_chunks = -(-m_pad // MCH)
    base = m_pad // n_chunks // 64 * 64
    sizes = [base] * n_chunks
    extra = m_pad - base * n_chunks
    i = 0
    while extra > 0:
        add = min(64, extra)
        sizes[i] += add
        extra -= add
        i = (i + 1) % n_chunks
    sizes.sort()
    if n_chunks >= 3:
        order = [sizes[0]] + sorted(sizes[2:], reverse=True) + [sizes[1]]
    else:
        order = sizes
    chunks = []
    m = 0
    for w in order:
        chunks.append((m, w))
        m += w
    assert m == m_pad and all(w <= MCH for _, w in chunks)
    return chunks


def _build(m_pad):
    import concourse.mybir as mybir
    import concourse.tile as tile
    from concourse import bacc

    if m_pad in _BUILD_CACHE:
        return _BUILD_CACHE[m_pad]

    nc = bacc.Bacc("TRN2", target_bir_lowering=False, debug=False,
                   num_devices=N_CORES)

    # Partition-major DRAM layouts: the DMA walks source and dest in
    # their own AP order, so SBUF [128, d1, d2] needs DRAM [128, d1, d2].
    # A is additionally chunk-major ([kc][w] packed per m-chunk) so one
    # chunk load is a [128, KC*w] descriptor with KC*w*2 (~6KB)
    # contiguous runs per partition — small-run descriptors measurably
    # crawl (~230GB/s) during the critical lead-in.
    at_d = nc.dram_tensor("at", [KP, KC * m_pad], mybir.dt.bfloat16,
                          kind="ExternalInput")
    bt_d = nc.dram_tensor("bt", [KP, NT, KC * KP], mybir.dt.bfloat16,
                          kind="ExternalInput")
    o_d = nc.dram_tensor("o", [KP, NT, m_pad], mybir.dt.bfloat16,
                         kind="ExternalOutput")

    chunks = _m_chunks(m_pad)
    n_ch = len(chunks)

    with tile.TileContext(nc) as tc:
        with (
            tc.tile_pool(name="ats", bufs=1) as ap,
            tc.tile_pool(name="bts", bufs=1) as bp,
            tc.tile_pool(name="wrm", bufs=1) as wp,
            tc.tile_pool(name="ost", bufs=4) as op,
            tc.tile_pool(name="ps", bufs=8, space="PSUM") as pp,
        ):
            at_s = ap.tile([KP, KC, m_pad], mybir.dt.bfloat16)
            bt_s = bp.tile([KP, NT, KC * KP], mybir.dt.bfloat16)

            # PE warmup: junk matmuls on a scratch tile run while the
            # input DMAs stream, so the HAM clock gate is released
            # before the first real matmul (~3.4us of PE busy needed).
            # gpsimd memset: gpsimd's engine preamble retires earliest,
            # so the junk chain starts ~1.3us sooner than off DVE.
            junk = wp.tile([KP, MCH], mybir.dt.bfloat16)
            nc.gpsimd.memset(junk[:], 0.0)
            wps = pp.tile([KP, MCH], mybir.dt.float32, name="wps",
                          tag="ps")
            for _ in range(N_WARMUP):
                nc.tensor.matmul(wps[:], junk[:, 0:KP], junk[:],
                                 start=True, stop=True)

            # Loads. Each ring drains FIFO, so emission order here is the
            # arrival schedule: first-chain prefix (bt[nt0], bt[nt1], the
            # two halves of A chunk 0) first, then B n-tile groups
            # interleaved across rings just ahead of the sweep, then the
            # remaining A chunks (not needed until chunk boundaries).
            (mc0, w0) = chunks[0]
            rings = [nc.sync, nc.scalar]
            c0 = KC * mc0
            nc.sync.dma_start(at_s[:, 0:4, mc0:mc0 + w0],
                              at_d[:, c0:c0 + 4 * w0])
            nc.scalar.dma_start(at_s[:, 4:8, mc0:mc0 + w0],
                                at_d[:, c0 + 4 * w0:c0 + 8 * w0])
            for g0, g1, r in BT_GROUPS:
                rings[r].dma_start(bt_s[:, g0:g1, :], bt_d[:, g0:g1, :])
            for ci, (mc, w) in enumerate(chunks[1:]):
                ring = nc.scalar if ci % 2 == 0 else nc.sync
                ring.dma_start(at_s[:, :, mc:mc + w],
                               at_d[:, KC * mc:KC * (mc + w)])

            # GEMM: chunk-outer, n-tile inner. One single-bank PSUM tile
            # per chain (8 cycling => ~10us of recycle slack) and one
            # cast per chain, alternating the two PSUM-reader engines —
            # shared multi-bank tiles made the scheduler serialize the
            # casts, stretching the post-stream drain. Stores are
            # per-quad on the SWDGE path (keeps HWDGE rings clear for
            # loads); the last chunk stores per-chain on the (idle by
            # then) rings, and the very last chain splits its cast and
            # store across both engines/rings to shorten the tail.
            for ci, (mc, w) in enumerate(chunks):
                last_chunk = ci == n_ch - 1
                for q in range(NT // 4):
                    ot = op.tile([KP, 4, w], mybir.dt.bfloat16, name="ot")
                    for j in range(4):
                        nt = 4 * q + j
                        p = pp.tile([KP, MCH], mybir.dt.float32, name="p",
                                    tag="ps")
                        for kc in range(KC):
                            nc.tensor.matmul(
                                p[:, 0:w],
                                bt_s[:, nt, kc * KP:(kc + 1) * KP],
                                at_s[:, kc, mc:mc + w],
                                start=(kc == 0),
                                stop=(kc == KC - 1),
                            )
                        last_chain = last_chunk and nt == NT - 1
                        if last_chain:
                            h = w // 2
                            nc.vector.tensor_copy(ot[:, j, 0:h], p[:, 0:h])
                            nc.scalar.copy(ot[:, j, h:w], p[:, h:w])
                            nc.sync.dma_start(
                                o_d[:, nt, mc:mc + h], ot[:, j, 0:h])
                            nc.scalar.dma_start(
                                o_d[:, nt, mc + h:mc + w], ot[:, j, h:w])
                        else:
                            if nt % 2 == 0:
                                nc.vector.tensor_copy(ot[:, j, :], p[:, 0:w])
                            else:
                                nc.scalar.copy(ot[:, j, :], p[:, 0:w])
                            if last_chunk:
                                st = nc.sync if nt % 2 == 0 else nc.scalar
                                st.dma_start(
                                    o_d[:, nt, mc:mc + w], ot[:, j, :])
                    if not last_chunk:
                        nc.gpsimd.dma_start(
                            o_d[:, 4 * q:4 * q + 4, mc:mc + w], ot[:])

    nc.compile()
    _BUILD_CACHE[m_pad] = nc
    return nc


SEC_CAP = 4096     # max rows one core takes in one launch (bounds SBUF use)


def _shard(m_indices):
    """Dispatch rows to (expert, row-subset) sections, <=8 per launch.

    In the common balanced case this is exactly one section per expert and
    a single launch. If one expert is so heavy that its section exceeds
    SEC_CAP, it is split into multiple sections (and, beyond 8 sections
    total, into multiple launches) so SBUF capacity is never exceeded.
    """
    g = np.where((m_indices >= 0) & (m_indices < G), m_indices, 0)
    rows = [np.nonzero(g == e)[0] for e in range(G)]
    sections = []                        # (expert, row_indices)
    for e in range(G):
        for s in range(0, max(len(rows[e]), 1), SEC_CAP):
            sections.append((e, rows[e][s:s + SEC_CAP]))
    sections.sort(key=lambda s: -len(s[1]))
    launches = [sections[i:i + N_CORES]
                for i in range(0, len(sections), N_CORES)]
    return launches


def _prep_in_maps(lhs, rhs, launch, m_pad):
    in_maps = []
    for slot in range(N_CORES):
        e, r = launch[slot] if slot < len(launch) else (0, [])
        a = np.zeros((m_pad, K), dtype=ml_dtypes.bfloat16)
        if len(r):
            a[:len(r)] = lhs[r]
        # [m, k] -> [kp, kc, m] -> chunk-major flat [kp, kc*m_pad]
        at3 = a.T.reshape(KC, KP, m_pad).transpose(1, 0, 2)
        at = np.concatenate(
            [at3[:, :, mc:mc + w].reshape(KP, KC * w)
             for (mc, w) in _m_chunks(m_pad)], axis=1)
        # [n, k] -> [kp, nt, kc*128]
        bt = rhs[e].T.reshape(KC, KP, NT, KP).transpose(1, 2, 0, 3) \
            .reshape(KP, NT, KC * KP)
        in_maps.append({
            "at": np.ascontiguousarray(at),
            "bt": np.ascontiguousarray(bt),
        })
    return in_maps


def kernel(lhs, rhs, m_indices):
    from concourse import bass_utils

    lhs = np.asarray(lhs)
    rhs = np.asarray(rhs)
    m_indices = np.asarray(m_indices)
    M = lhs.shape[0]

    out = np.zeros((M, N), dtype=ml_dtypes.bfloat16)
    for launch in _shard(m_indices):
        m_pad = max(-(-max(len(r) for _, r in launch) // 64) * 64, 128)
        nc = _build(m_pad)
        in_maps = _prep_in_maps(lhs, rhs, launch, m_pad)
        res = bass_utils.run_bass_kernel_spmd(
            nc, in_maps, core_ids=list(range(N_CORES)))
        for slot, (e, r) in enumerate(launch):
            if len(r):
                o = res.results[slot]["o"]       # [kp, nt, m_pad]
                oT = o.transpose(1, 0, 2).reshape(N, m_pad)
                out[r] = oT[:, :len(r)].T
    return out


# revision 16
# speedup vs baseline: 1.1941x; 1.1941x over previous
"""Grouped (MoE-routed) GEMM on 8 Trainium2 NeuronCores.

out[m, n] = sum_k lhs[m, k] * rhs[g[m], n, k],  g = clamp(m_indices, 0, G)

Strategy: expert-parallel. Host dispatches rows by m_indices (the
"all-to-all" is a host-side gather since we hold full inputs), core c gets
expert c's rows padded to a common M_pad, plus expert c's weight matrix.
Every core runs one identical dense GEMM program computing the transposed
output  oT[N, M_pad] = B[N, K] @ A[M_pad, K]^T  (bf16 in, fp32 acc).

v2 layout/ordering notes (driven by the v1 NTFF trace):
- All DRAM operands are partition-major so every load/store is ONE big
  3D descriptor (256KB-1MB). v1's 64-114KB descriptors serialized on the
  two HWDGE rings at ~150-250GB/s and starved the PE twice (a 3.4us idle
  that also re-throttled the HAM clock gate, costing ~5us more).
- B is nt-major in DRAM so B n-tile groups arrive in exactly the order
  the (m-chunk outer, n-tile inner) chain sweep consumes them.
- PSUM is managed as two 4-bank tiles [128, 4, 512]f32; a quad of 4
  chains shares one tile, then a single DVE/ACT cast moves all 4 banks
  to SBUF and a single DMA stores them. This quarters the cast/store
  instruction count: the end-of-NEFF teardown clears every semaphore
  ever allocated, one EVENT_SEMAPHORE each (~110ns serialized/engine),
  so instruction/semaphore count is ~6us of the measured window.
- The final quad casts/stores per-chain on the (idle by then) HWDGE
  rings to minimize the drain tail after the last matmul.
- 8 junk matmuls on a scratch tile lift the HAM clock gate (1.2->2.4GHz)
  during the load lead-in so the first real chain runs warm.
"""

import numpy as np
import ml_dtypes

K = 1024
N = 2048
G = 8
N_CORES = 8
KP = 128           # SBUF partitions / contraction chunk
KC = K // KP       # 8 k-chunks
NT = N // KP       # 16 stationary n-tiles
MCH = 512          # max moving-operand m-chunk (one PSUM bank of fp32)
N_WARMUP = 12      # dummy matmuls to lift the HAM clock gate

# B n-tile load groups with explicit ring assignment (0=sync, 1=scalar).
# Each HWDGE ring drains its descriptors FIFO at ~179GB/s (the two rings
# split the ~358GB/s HBM-per-core limit), so this list IS the flow
# control: a group must sit early enough in its ring that it lands
# before the (m-chunk outer, nt inner) sweep reaches it. Singles first
# (latency), pairs after (efficiency).
BT_GROUPS = [(0, 1, 0), (1, 2, 1), (2, 3, 0), (3, 4, 1),
             (4, 6, 0), (6, 8, 1), (8, 10, 0), (10, 12, 1),
             (12, 14, 0), (14, 16, 1)]

_BUILD_CACHE = {}


def _m_chunks(m_pad):
    """Near-equal 64-aligned chunks of <=512, smallest first and last.

    Small first chunk -> the first chain needs the least data in SBUF;
    small last chunk -> the final casts/stores drain quickly.
    """
    n_chunks = -(-m_pad // MCH)
    base = m_pad // n_chunks // 64 * 64
    sizes = [base] * n_chunks
    extra = m_pad - base * n_chunks
    i = 0
    while extra > 0:
        add = min(64, extra)
        sizes[i] += add
        extra -= add
        i = (i + 1) % n_chunks
    sizes.sort()
    if n_chunks >= 3:
        order = [sizes[0]] + sorted(sizes[2:], reverse=True) + [sizes[1]]
    else:
        order = sizes
    chunks = []
    m = 0
    for w in order:
        chunks.append((m, w))
        m += w
    assert m == m_pad and all(w <= MCH for _, w in chunks)
    return chunks


def _build(m_pad):
    import concourse.mybir as mybir
    import concourse.tile as tile
    from concourse import bacc

    if m_pad in _BUILD_CACHE:
        return _BUILD_CACHE[m_pad]

    nc = bacc.Bacc("TRN2", target_bir_lowering=False, debug=False,
                   num_devices=N_CORES)

    # Partition-major DRAM layouts: the DMA walks source and dest in
    # their own AP order, so SBUF [128, d1, d2] needs DRAM [128, d1, d2].
    # A is additionally chunk-major ([kc][w] packed per m-chunk) so one
    # chunk load is a [128, KC*w] descriptor with KC*w*2 (~6KB)
    # contiguous runs per partition — small-run descriptors measurably
    # crawl (~230GB/s) during the critical lead-in.
    at_d = nc.dram_tensor("at", [KP, KC * m_pad], mybir.dt.bfloat16,
                          kind="ExternalInput")
    bt_d = nc.dram_tensor("bt", [KP, NT, KC * KP], mybir.dt.bfloat16,
                          kind="ExternalInput")
    o_d = nc.dram_tensor("o", [KP, NT, m_pad], mybir.dt.bfloat16,
                         kind="ExternalOutput")

    chunks = _m_chunks(m_pad)
    n_ch = len(chunks)

    with tile.TileContext(nc) as tc:
        with (
            tc.tile_pool(name="ats", bufs=1) as ap,
            tc.tile_pool(name="bts", bufs=1) as bp,
            tc.tile_pool(name="wrm", bufs=1) as wp,
            tc.tile_pool(name="ost", bufs=4) as op,
            tc.tile_pool(name="ps", bufs=8, space="PSUM") as pp,
        ):
            at_s = ap.tile([KP, KC, m_pad], mybir.dt.bfloat16)
            bt_s = bp.tile([KP, NT, KC * KP], mybir.dt.bfloat16)

            # PE warmup: junk matmuls on a scratch tile run while the
            # input DMAs stream, so the HAM clock gate is released
            # before the first real matmul (~3.4us of PE busy needed).
            # gpsimd memset: gpsimd's engine preamble retires earliest,
            # so the junk chain starts ~1.3us sooner than off DVE.
            junk = wp.tile([KP, MCH], mybir.dt.bfloat16)
            nc.gpsimd.memset(junk[:], 0.0)
            wps = pp.tile([KP, MCH], mybir.dt.float32, name="wps",
                          tag="ps")
            for _ in range(N_WARMUP):
                nc.tensor.matmul(wps[:], junk[:, 0:KP], junk[:],
                                 start=True, stop=True)

            # Loads. Each ring drains FIFO, so emission order here is the
            # arrival schedule: first-chain prefix (bt[nt0], bt[nt1], the
            # two halves of A chunk 0) first, then B n-tile groups
            # interleaved across rings just ahead of the sweep, then the
            # remaining A chunks (not needed until chunk boundaries).
            (mc0, w0) = chunks[0]
            rings = [nc.sync, nc.scalar]
            c0 = KC * mc0
            nc.sync.dma_start(at_s[:, 0:4, mc0:mc0 + w0],
                              at_d[:, c0:c0 + 4 * w0])
            nc.scalar.dma_start(at_s[:, 4:8, mc0:mc0 + w0],
                                at_d[:, c0 + 4 * w0:c0 + 8 * w0])
            for g0, g1, r in BT_GROUPS:
                rings[r].dma_start(bt_s[:, g0:g1, :], bt_d[:, g0:g1, :])
            for ci, (mc, w) in enumerate(chunks[1:]):
                ring = nc.scalar if ci % 2 == 0 else nc.sync
                ring.dma_start(at_s[:, :, mc:mc + w],
                               at_d[:, KC * mc:KC * (mc + w)])

            # GEMM: chunk-outer, n-tile inner. One single-bank PSUM tile
            # and one DVE cast + one store per chain. All mid-kernel
            # casts stay on DVE: ACT casts interleaved with the stream
            # measurably slowed every matmul (~25ns/MM, +13us total) —
            # only the very last chain splits its cast across DVE+ACT
            # (stream is over by then) to shorten the drain. Bulk stores
            # ride the otherwise-idle SWDGE path so the HWDGE rings stay

            # clear for loads; the last chunk goes out via the (by then
            # idle) rings so the SWDGE queue drain overlaps compute.
            for ci, (mc, w) in enumerate(chunks):
                last_chunk = ci == n_ch - 1
                for nt in range(NT):
                    p = pp.tile([KP, w], mybir.dt.float32, name="p",
                                tag="ps")
                    for kc in range(KC):
                        nc.tensor.matmul(
                            p[:],
                            bt_s[:, nt, kc * KP:(kc + 1) * KP],
                            at_s[:, kc, mc:mc + w],
                            start=(kc == 0),
                            stop=(kc == KC - 1),
                        )
                    ot = op.tile([KP, w], mybir.dt.bfloat16, name="ot")
                    if last_chunk and nt == NT - 1:
                        h = w // 2
                        nc.vector.tensor_copy(ot[:, 0:h], p[:, 0:h])
                        nc.scalar.copy(ot[:, h:w], p[:, h:w])
                        nc.sync.dma_start(
                            o_d[:, nt, mc:mc + h], ot[:, 0:h])
                        nc.scalar.dma_start(
                            o_d[:, nt, mc + h:mc + w], ot[:, h:w])
                    else:
                        nc.vector.tensor_copy(ot[:], p[:])
                        if last_chunk:
                            st = nc.sync if nt % 2 == 0 else nc.scalar
                        else:
                            st = nc.gpsimd
                        st.dma_start(o_d[:, nt, mc:mc + w], ot[:])

    nc.compile()
    _BUILD_CACHE[m_pad] = nc
    return nc


SEC_CAP = 4096     # max rows one core takes in one launch (bounds SBUF use)


def _shard(m_indices):
    """Dispatch rows to (expert, row-subset) sections, <=8 per launch.

    In the common balanced case this is exactly one section per expert and
    a single launch. If one expert is so heavy that its section exceeds
    SEC_CAP, it is split into multiple sections (and, beyond 8 sections
    total, into multiple launches) so SBUF capacity is never exceeded.
    """
    g = np.where((m_indices >= 0) & (m_indices < G), m_indices, 0)
    rows = [np.nonzero(g == e)[0] for e in range(G)]
    sections = []                        # (expert, row_indices)
    for e in range(G):
        for s in range(0, max(len(rows[e]), 1), SEC_CAP):
            sections.append((e, rows[e][s:s + SEC_CAP]))
    sections.sort(key=lambda s: -len(s[1]))
    launches = [sections[i:i + N_CORES]
                for i in range(0, len(sections), N_CORES)]
    return launches


def _prep_in_maps(lhs, rhs, launch, m_pad):
    in_maps = []
    for slot in range(N_CORES):
        e, r = launch[slot] if slot < len(launch) else (0, [])
        a = np.zeros((m_pad, K), dtype=ml_dtypes.bfloat16)
        if len(r):
            a[:len(r)] = lhs[r]
        # [m, k] -> [kp, kc, m] -> chunk-major flat [kp, kc*m_pad]
        at3 = a.T.reshape(KC, KP, m_pad).transpose(1, 0, 2)
        at = np.concatenate(
            [at3[:, :, mc:mc + w].reshape(KP, KC * w)
             for (mc, w) in _m_chunks(m_pad)], axis=1)
        # [n, k] -> [kp, nt, kc*128]
        bt = rhs[e].T.reshape(KC, KP, NT, KP).transpose(1, 2, 0, 3) \
            .reshape(KP, NT, KC * KP)
        in_maps.append({
            "at": np.ascontiguousarray(at),
            "bt": np.ascontiguousarray(bt),
        })
    return in_maps


def kernel(lhs, rhs, m_indices):
    from concourse import bass_utils

    lhs = np.asarray(lhs)
    rhs = np.asarray(rhs)
    m_indices = np.asarray(m_indices)
    M = lhs.shape[0]

    out = np.zeros((M, N), dtype=ml_dtypes.bfloat16)
    for launch in _shard(m_indices):
        m_pad = max(-(-max(len(r) for _, r in launch) // 64) * 64, 128)
        nc = _build(m_pad)
        in_maps = _prep_in_maps(lhs, rhs, launch, m_pad)
        res = bass_utils.run_bass_kernel_spmd(
            nc, in_maps, core_ids=list(range(N_CORES)))
        for slot, (e, r) in enumerate(launch):
            if len(r):
                o = res.results[slot]["o"]       # [kp, nt, m_pad]
                oT = o.transpose(1, 0, 2).reshape(N, m_pad)
                out[r] = oT[:, :len(r)].T
    return out
